# revision 45
# baseline (speedup 1.0000x reference)
"""Trainium2 Bass kernel for DAS (delay-and-sum) ultrasound beamforming.

Math: the per-(t,e,z) delay/phase depend on (t,e) only through
vx = gx[t]-ex[e], i.e. on delta = t-e (Toeplitz geometry). Per-delta tables
(gather index i0, fused interp/rotation/apod weights) are computed on host
from the small geometry inputs; the sample data is processed on 8 NeuronCores.

The dominant cost is the ~45 MB/s axon tunnel, not device compute, so the
design minimizes bytes on the wire (~580 MB for the naive layout -> ~44 MB):
  * only "active" deltas are shipped/processed - deltas whose apodization
    mask never accepts any depth contribute exactly zero (|delta|>99 here).
  * per-delta sample windows - only samples in [lo_d, hi_d] (the range
    reachable with nonzero weight) are shipped (~22% of each row).
  * int8 sample data (DATA_INT8): samples go over the wire as
    (I[2p], Q[2p], I[2p+1], Q[2p+1]) int8 quadruples with a per-row scale,
    gathered at pair indices i0>>1 and (i0+1)>>1, dequantized per row after
    the gather (ACT engine, per-partition scale); which of the two lanes
    holds each interp tap (the parity of i0) is folded into the host-built
    weight tables. Quantization adds ~8e-3 rel error (tolerance 2e-2).
  * 8 weight columns instead of 16: the accQ coefficients are +-the accI
    columns of the partner lane, handled by an add/subtract accumulate op.
  * index tables shipped in the compact 16-partition wrapped form and
    replicated to 128 partitions on device with 3 doubling DMA copies.
  * all per-call bytes ride in ONE flat int8 tensor (per-slot row blocks
    + fp16 dequant scales, viewed via AP rearrange/bitcast) - the tunnel
    charges ~10ms per array, so 27 inputs -> 1 saves ~250ms/call.
  * on-device AllReduce (OUTPUT_RS) of the per-core partial sums with a
    replicated shard_map output spec: the host fetches one summed replica
    in a single transfer (a sharded output costs ~90ms in per-shard
    fetch roundtrips).
  * a cached jitted PJRT executable (no per-call retrace) with
    device-persistent zero output buffers and geometry tables (idx/wts).

Per (core, slot) = one delta diagonal: DMA the windowed diagonal rows ->
GPSIMD ap_gather at the two pair indices -> dequant -> PE transpose of each
lane to [z, t] -> DVE/ACT multiply by per-delta weight columns (free-axis
broadcast) and accumulate in f32 -> fp16 -> AllReduce across cores.
Host reassembles [T, Z] and applies sparse mask corrections (zero for this
geometry).
"""
import os
import sys

for _p in ('/opt/trn_rl_repo', '/root/.axon_site/_ro/trn_rl_repo'):
    if os.path.isdir(_p) and _p not in sys.path:
        sys.path.append(_p)

import numpy as np

T, E, S, Z = 128, 128, 4096, 2048
PI = 3.14159265359
MIN_WIDTH = 0.001
N_CORES = 8
NBLK = 16          # z blocks of 128
DUMMY = 999
# int8 sample data on the wire (halves the dominant transfer): samples are
# shipped as (I[2p], Q[2p], I[2p+1], Q[2p+1]) int8 quadruples gathered at
# pair indices, dequantized per row after the gather; the tap parity
# selection is folded into the host-built weight tables. False = fp16 wire.
DATA_INT8 = True
# on-device AllReduce of the per-core partial sums; the output is declared
# replicated so the host fetches one summed copy in a single transfer.
OUTPUT_RS = True


def _f32(x):
    return np.asarray(x, dtype=np.float32)


# ---------------------------------------------------------------- host math
def compute_tables(grid, tx_ori, ele_pos, time_zero, fs, c, fdemod, rxfnum):
    grid = _f32(grid); tx_ori = _f32(tx_ori); ele_pos = _f32(ele_pos)
    time_zero = _f32(time_zero)
    gx = grid[:, 0, 0]
    zax = grid[0, :, 2]
    ex = ele_pos[:, 0]

    vx_te = (gx[:, None] - ex[None, :]).astype(np.float32)
    vz = zax.astype(np.float32)
    with np.errstate(divide='ignore', invalid='ignore'):
        ratio = np.abs(vz[None, None, :] / vx_te[:, :, None])
    m = ratio > np.float32(rxfnum)
    m |= (np.abs(vx_te) <= np.float32(MIN_WIDTH))[:, :, None]
    m |= ((vx_te >= np.float32(MIN_WIDTH)) & (gx[:, None] <= ex[0]))[:, :, None]
    m |= ((vx_te <= np.float32(-MIN_WIDTH)) & (gx[:, None] >= ex[-1]))[:, :, None]
    mask_exact = m

    d3 = grid - tx_ori[:, None, :]
    txdel = np.sqrt((d3 * d3).sum(-1, dtype=np.float32)).astype(np.float32)

    nd = 255
    i0_tab = np.zeros((nd, Z), np.int32)
    frac_tab = np.zeros((nd, Z), np.float32)
    ct_tab = np.zeros((nd, Z), np.float32)
    st_tab = np.zeros((nd, Z), np.float32)
    v0_tab = np.zeros((nd, Z), np.float32)
    v1_tab = np.zeros((nd, Z), np.float32)
    mask_tab = np.zeros((nd, Z), bool)
    active = np.zeros(nd, bool)
    lo_tab = np.zeros(nd, np.int32)
    hi_tab = np.zeros(nd, np.int32)
    for delta in range(-127, 128):
        t_rep = max(0, delta); e_rep = t_rep - delta
        vx = vx_te[t_rep, e_rep]
        rx = np.sqrt(vx * vx + vz * vz).astype(np.float32)
        delays = ((txdel[t_rep] + rx) / np.float32(c)
                  + time_zero[t_rep]) * np.float32(fs)
        i0f = np.floor(delays)
        frac = (delays - i0f).astype(np.float32)
        i0 = i0f.astype(np.int32)
        tshift = delays / np.float32(fs) - zax * np.float32(2.0) / np.float32(c)
        theta = (np.float32(2.0 * PI * fdemod) * tshift).astype(np.float32)
        j = delta + 127
        i0_tab[j] = i0
        frac_tab[j] = frac
        ct_tab[j] = np.cos(theta, dtype=np.float32)
        st_tab[j] = np.sin(theta, dtype=np.float32)
        v0 = (i0 >= 0) & (i0 < S)
        v1 = (i0 + 1 >= 0) & (i0 + 1 < S)
        v0_tab[j] = v0
        v1_tab[j] = v1
        mask_tab[j] = mask_exact[t_rep, e_rep]
        # used-sample window: depths where the apod mask accepts AND at
        # least one interp tap is a real sample
        used = mask_tab[j] & (v0 | v1)
        active[j] = used.any()
        if active[j]:
            tap_lo = np.where(v0, i0, i0 + 1)         # first valid tap
            tap_hi = np.where(v1, i0 + 1, i0)         # last valid tap
            lo_tab[j] = int(tap_lo[used].min())
            hi_tab[j] = int(tap_hi[used].max())
    return dict(i0=i0_tab, frac=frac_tab, ct=ct_tab, st=st_tab,
                v0=v0_tab, v1=v1_tab, mask_tab=mask_tab,
                mask_exact=mask_exact, active=active, lo=lo_tab, hi=hi_tab)


def build_weight_tables(tabs):
    """[255, 6, Z]: (wa, wb, -wc, -wd, wc, wd);
    accI += wa*I0 + wb*I1 - wc*Q0 - wd*Q1
    accQ += wc*I0 + wd*I1 + wa*Q0 + wb*Q1"""
    apod = tabs['mask_tab'].astype(np.float32)
    omf = np.float32(1.0) - tabs['frac']
    wa = apod * tabs['ct'] * omf * tabs['v0']
    wb = apod * tabs['ct'] * tabs['frac'] * tabs['v1']
    wc = apod * tabs['st'] * omf * tabs['v0']
    wd = apod * tabs['st'] * tabs['frac'] * tabs['v1']
    return np.stack([wa, wb, -wc, -wd, wc, wd], axis=1).astype(np.float32)


def _win(tabs, d):
    """Per-delta shipped window [lo, hi]; lo is even in int8 (pair) mode."""
    j = d + 127
    lo = int(tabs['lo'][j]); hi = int(tabs['hi'][j])
    if DATA_INT8:
        lo -= lo & 1
    return lo, hi


def build_slots(tabs):
    """Group active deltas into SPMD-uniform slots of 8 (one per core),
    same sign and adjacent |delta| so window/extent padding stays small.
    'W' is the per-slot gather element count: samples (fp16 mode) or
    sample pairs (int8 mode)."""
    act = [d for d in range(-127, 128) if tabs['active'][d + 127]]
    pos = sorted([d for d in act if d >= 0], reverse=True)
    neg = sorted([d for d in act if d < 0])          # descending |d|
    slots = []
    for group_src, is_pos in ((pos, True), (neg, False)):
        # dummies go in the first (largest |delta|) chunk, where the slot
        # tensor is smallest - a dummy core ships a whole [ext, W] block of
        # zeros, so parking dummies in an ext=128 chunk wastes ~370KB each
        npad = (-len(group_src)) % 8
        group_src = [DUMMY] * npad + list(group_src)
        for i in range(0, len(group_src), 8):
            g = [d for d in group_src[i:i + 8] if d != DUMMY]
            if DATA_INT8:
                W = max((lambda lo, hi: (hi - lo + 2) // 2)(*_win(tabs, d))
                        for d in g)
                W = (W + 3) & ~3
            else:
                W = max((lambda lo, hi: hi - lo + 1)(*_win(tabs, d))
                        for d in g)
                W = (W + 7) & ~7
            toff = min(g) if is_pos else 0
            ext = 128 - min(abs(d) for d in g)
            slots.append(dict(deltas=[DUMMY] * (8 - len(g)) + g,
                              toff=toff, ext=ext, W=W))
    return slots


def build_weight_tables16(tabs):
    """int8 pair mode: [255, 8, Z] weight columns, tap parity folded in.
    Sources s=0..7 are (A0=Ieven, A1=Qeven, A2=Iodd, A3=Qodd, B0..B3);
    tab s drives accI (always add). The accQ coefficients are +-these same
    columns: accQ uses tab s^1, added for odd s and subtracted for even s
    (accQ(A0) = wc(1-e0) = -tab1, accQ(A1) = wa(1-e0) = tab0, ...)."""
    apod = tabs['mask_tab'].astype(np.float32)
    omf = np.float32(1.0) - tabs['frac']
    wa = apod * tabs['ct'] * omf * tabs['v0']
    wb = apod * tabs['ct'] * tabs['frac'] * tabs['v1']
    wc = apod * tabs['st'] * omf * tabs['v0']
    wd = apod * tabs['st'] * tabs['frac'] * tabs['v1']
    out = np.zeros((255, 8, Z), np.float32)
    for d in range(-127, 128):
        j = d + 127
        if not tabs['active'][j]:
            continue
        lo, hi = _win(tabs, d)
        i0c = np.clip(tabs['i0'][j], lo, hi)
        i1c = np.clip(tabs['i0'][j] + 1, lo, hi)
        e0 = ((i0c - lo) & 1).astype(np.float32)
        e1 = ((i1c - lo) & 1).astype(np.float32)
        a, b = 1.0 - e0, e0
        c_, dd = 1.0 - e1, e1
        out[j, 0] = wa[j] * a;   out[j, 1] = -wc[j] * a
        out[j, 2] = wa[j] * b;   out[j, 3] = -wc[j] * b
        out[j, 4] = wb[j] * c_;  out[j, 5] = -wd[j] * c_
        out[j, 6] = wb[j] * dd;  out[j, 7] = -wd[j] * dd
    return out


def corrections(idata, qdata, tabs):
    corrI = np.zeros((T, Z), np.float32)
    corrQ = np.zeros((T, Z), np.float32)
    i0c = np.clip(tabs['i0'], 0, S - 1)
    i1c = np.clip(tabs['i0'] + 1, 0, S - 1)
    for delta in range(-127, 128):
        j = delta + 127
        ts = np.arange(max(0, delta), min(T - 1, T - 1 + delta) + 1)
        es = ts - delta
        dm = (tabs['mask_exact'][ts, es, :].astype(np.int8)
              - tabs['mask_tab'][j][None, :].astype(np.int8))
        nz = np.argwhere(dm != 0)
        if nz.size == 0:
            continue
        ti, zi = nz[:, 0], nz[:, 1]
        tt, ee = ts[ti], es[ti]
        sgn = dm[ti, zi].astype(np.float32)
        f = tabs['frac'][j][zi]; ct = tabs['ct'][j][zi]; st = tabs['st'][j][zi]
        v0 = tabs['v0'][j][zi]; v1 = tabs['v1'][j][zi]
        I0 = idata[tt, ee, i0c[j][zi]] * v0; I1 = idata[tt, ee, i1c[j][zi]] * v1
        Q0 = qdata[tt, ee, i0c[j][zi]] * v0; Q1 = qdata[tt, ee, i1c[j][zi]] * v1
        fi = (1 - f) * I0 + f * I1
        fq = (1 - f) * Q0 + f * Q1
        np.add.at(corrI, (tt, zi), sgn * (ct * fi - st * fq))
        np.add.at(corrQ, (tt, zi), sgn * (ct * fq + st * fi))
    return corrI, corrQ


# ------------------------------------------------------------- bass program
_CACHE = {}


def _build_program(slots):
    import concourse.bacc as bacc
    import concourse.mybir as mybir
    from concourse.tile import TileContext
    from concourse.masks import make_identity

    F16 = mybir.dt.float16
    F32 = mybir.dt.float32
    I8 = mybir.dt.int8
    NS = len(slots)
    Wmax = max(sl['W'] for sl in slots)
    DLANES = 4 if DATA_INT8 else 2      # int8 pair quads vs fp16 I/Q pairs
    DDT = I8 if DATA_INT8 else F16
    NTAB = 8 if DATA_INT8 else 6
    nc = bacc.Bacc("TRN2", target_bir_lowering=False, debug=False,
                   num_devices=N_CORES)
    if DATA_INT8:
        # all per-call data rides in ONE flat int8 tensor (the tunnel pays a
        # per-array overhead, so 27 inputs -> 1): per-slot row blocks, then
        # the per-slot fp16 dequant scales, viewed via AP rearrange/bitcast
        offs = [0]
        for sl in slots:
            offs.append(offs[-1] + sl['ext'] * sl['W'] * 4)
        scl_off = offs[-1]
        TOTAL = scl_off + NS * 256
        blob_d = nc.dram_tensor("blob", [TOTAL], I8,
                                kind="ExternalInput").ap()
    else:
        rows_d = [nc.dram_tensor(f"rows{k:02d}",
                                 [sl['ext'], sl['W'] * DLANES],
                                 DDT, kind="ExternalInput").ap()
                  for k, sl in enumerate(slots)]
    idx_d = nc.dram_tensor("idx", [NS, 16, 256], mybir.dt.int16,
                           kind="ExternalInput").ap()
    wts_d = nc.dram_tensor("wts", [NS, 128, NTAB * 16], F16,
                           kind="ExternalInput").ap()
    if OUTPUT_RS:
        # AllReduce (not ReduceScatter): every core ends with the full sum,
        # so the host fetches ONE replica in one transfer - the per-shard
        # fetch overhead of a sharded output costs ~90ms for 8x128KB
        acc_d = nc.dram_tensor("acc", [256, Z], F16,
                               kind="ExternalOutput").ap()
        # collectives can't touch I/O tensors directly -> bounce buffers
        ccin = [nc.dram_tensor(n, [128, Z], F16).ap()
                for n in ("ccIin", "ccQin")]
        ccout = [nc.dram_tensor(n, [128, Z], F16).ap()
                 for n in ("ccIout", "ccQout")]
    else:
        accI_d = nc.dram_tensor("accI", [128, Z], F16,
                                kind="ExternalOutput").ap()
        accQ_d = nc.dram_tensor("accQ", [128, Z], F16,
                                kind="ExternalOutput").ap()

    with TileContext(nc) as tc:
        with tc.tile_pool(name="data", bufs=2) as dpool, \
             tc.tile_pool(name="gout", bufs=2) as gpool, \
             tc.tile_pool(name="small", bufs=2) as spool, \
             tc.tile_pool(name="tmp", bufs=3) as tpool, \
             tc.tile_pool(name="accp", bufs=1) as apool, \
             tc.tile_pool(name="psum", bufs=2, space="PSUM") as ppool:
            ident = apool.tile([128, 128], F16, tag="ident")
            make_identity(nc, ident[:])
            accI = apool.tile([128, NBLK, 128], F32, tag="accI")
            accQ = apool.tile([128, NBLK, 128], F32, tag="accQ")
            nc.vector.memset(accI[:], 0.0)
            nc.vector.memset(accQ[:], 0.0)

            for k, sl in enumerate(slots):
                ext, toff, W = sl['ext'], sl['toff'], sl['W']
                data_t = dpool.tile([128, Wmax, DLANES], DDT, tag="data")
                if ext < 128:
                    # the gather reads all 128 partitions; stale SBUF in
                    # [ext:128) would poison the PE transpose. Partition
                    # offsets are restricted (0/32/64/96 with limited
                    # extents), so clear the whole tile; the data DMA
                    # below overwrites [0:ext] afterwards.
                    nc.vector.memset(data_t[:, 0:W, :], 0.0)
                if DATA_INT8:
                    rows_ap = blob_d[offs[k]:offs[k + 1]] \
                        .rearrange('(p f) -> p f', p=ext)
                    nc.sync.dma_start(out=data_t[0:ext, 0:W, :], in_=rows_ap)
                else:
                    nc.sync.dma_start(out=data_t[0:ext, 0:W, :],
                                      in_=rows_d[k][:])
                idx_t = spool.tile([128, 256], mybir.dt.int16, tag="idx")
                nc.sync.dma_start(out=idx_t[0:16], in_=idx_d[k])
                nc.sync.dma_start(out=idx_t[16:32], in_=idx_t[0:16])
                nc.sync.dma_start(out=idx_t[32:64], in_=idx_t[0:32])
                nc.sync.dma_start(out=idx_t[64:128], in_=idx_t[0:64])
                w_t = spool.tile([128, NTAB * 16], F16, tag="wts")
                nc.sync.dma_start(out=w_t[:], in_=wts_d[k])

                gout0 = gpool.tile([128, Z, DLANES], DDT, tag="g0")
                gout1 = gpool.tile([128, Z, DLANES], DDT, tag="g1")
                nc.gpsimd.ap_gather(gout0[:], data_t[:, 0:W, :],
                                    idx_t[:, 0:128], channels=128,
                                    num_elems=W, d=DLANES, num_idxs=Z)
                nc.gpsimd.ap_gather(gout1[:], data_t[:, 0:W, :],
                                    idx_t[:, 128:256], channels=128,
                                    num_elems=W, d=DLANES, num_idxs=Z)

                ADD, SUB = mybir.AluOpType.add, mybir.AluOpType.subtract
                if DATA_INT8:
                    scl_ap = blob_d[scl_off + k * 256:scl_off + (k + 1) * 256] \
                        .rearrange('(p f) -> p f', p=128).bitcast(F16)
                    scl16 = spool.tile([128, 1], F16, tag="scl16")
                    nc.sync.dma_start(out=scl16[:], in_=scl_ap)
                    scl_t = spool.tile([128, 1], F32, tag="scl")
                    nc.scalar.copy(out=scl_t[:], in_=scl16[:])
                    deq0 = gpool.tile([128, Z, 4], F16, tag="d0")
                    deq1 = gpool.tile([128, Z, 4], F16, tag="d1")
                    nc.scalar.mul(out=deq0[:], in_=gout0[:],
                                  mul=scl_t[:, 0:1])
                    nc.scalar.mul(out=deq1[:], in_=gout1[:],
                                  mul=scl_t[:, 0:1])
                    # (source tile, lane, accI tab, accQ tab, accQ op):
                    # accQ coefficient for source s is +-tab (s^1) - see
                    # build_weight_tables16
                    lanes = [(deq0, l, l, l ^ 1, SUB if l % 2 == 0 else ADD)
                             for l in range(4)] + \
                            [(deq1, l, 4 + l, 4 + (l ^ 1),
                              SUB if l % 2 == 0 else ADD) for l in range(4)]
                else:
                    lanes = [(gout0, 0, 0, 4, ADD), (gout1, 0, 1, 5, ADD),
                             (gout0, 1, 2, 0, ADD), (gout1, 1, 3, 1, ADD)]

                for (src, ch, tabI, tabQ, qop) in lanes:
                    big = ppool.tile([128, NBLK, 128], F16, space="PSUM",
                                     tag="big")
                    for blk in range(NBLK):
                        nc.tensor.transpose(
                            out=big[:, blk, :],
                            in_=src[:, blk * 128:(blk + 1) * 128, ch],
                            identity=ident[:])
                    for (acc, tab, op2) in ((accI, tabI, ADD),
                                            (accQ, tabQ, qop)):
                        w_ap = w_t[:, tab * 16:(tab + 1) * 16] \
                            .broadcast_to([128, NBLK, ext])
                        tmp = tpool.tile([128, NBLK, 128], F32, tag="tmp")
                        nc.any.tensor_tensor(
                            out=tmp[:, :, 0:ext], in0=big[:, :, 0:ext],
                            in1=w_ap, op=mybir.AluOpType.mult)
                        nc.any.tensor_tensor(
                            out=acc[:, :, toff:toff + ext],
                            in0=acc[:, :, toff:toff + ext],
                            in1=tmp[:, :, 0:ext], op=op2)

            outI = apool.tile([128, Z], F16, tag="outI")
            outQ = apool.tile([128, Z], F16, tag="outQ")
            nc.scalar.copy(out=outI[:], in_=accI[:])
            nc.scalar.copy(out=outQ[:], in_=accQ[:])
            if OUTPUT_RS:
                groups = [list(range(N_CORES))]
                for i, (tile, cin, cout) in enumerate(
                        ((outI, ccin[0], ccout[0]), (outQ, ccin[1],
                                                     ccout[1]))):
                    nc.sync.dma_start(out=cin[:], in_=tile[:])
                    nc.gpsimd.collective_compute(
                        "AllReduce", mybir.AluOpType.add,
                        replica_groups=groups, ins=[cin[:]], outs=[cout[:]])
                    nc.sync.dma_start(out=acc_d[i * 128:(i + 1) * 128],
                                      in_=cout[:])
            else:
                nc.sync.dma_start(out=accI_d[:], in_=outI[:])
                nc.sync.dma_start(out=accQ_d[:], in_=outQ[:])
    nc.compile()
    return nc


# ------------------------------------------------------- cached PJRT runner
class _Runner:
    """Executes the compiled Bass module on 8 axon cores via a single
    cached jitted shard_map call (mirrors bass2jax.run_bass_via_pjrt, minus
    the per-call retrace, minus donation: outputs are fully written by the
    NEFF, so persistent device-resident zero buffers are passed instead of
    shipping fresh zeros every call)."""

    def __init__(self, nc):
        import jax
        import concourse.mybir as mybir
        from concourse import bass2jax as B
        from jax.sharding import Mesh, PartitionSpec
        from jax.experimental.shard_map import shard_map

        B.install_neuronx_cc_hook()
        assert nc.dbg_addr is None

        in_names, out_names, out_avals, zero_outs = [], [], [], []
        partition_name = (nc.partition_id_tensor.name
                          if nc.partition_id_tensor else None)
        for alloc in nc.m.functions[0].allocations:
            if not isinstance(alloc, mybir.MemoryLocationSet):
                continue
            name = alloc.memorylocations[0].name
            if alloc.kind == "ExternalInput":
                if name != partition_name:
                    in_names.append(name)
            elif alloc.kind == "ExternalOutput":
                shape = tuple(alloc.tensor_shape)
                dtype = mybir.dt.np(alloc.dtype)
                out_names.append(name)
                out_avals.append(jax.core.ShapedArray(shape, dtype))
                zero_outs.append(np.zeros((N_CORES * shape[0], *shape[1:]),
                                          dtype))
        n_params = len(in_names)
        all_names = list(in_names) + list(out_names)
        if partition_name is not None:
            all_names.append(partition_name)
        self.in_names = in_names
        self.out_names = out_names
        self.out_avals = out_avals

        def _body(*args):
            operands = list(args)
            if partition_name is not None:
                operands.append(B.partition_id_tensor())
            outs = B._bass_exec_p.bind(
                *operands,
                out_avals=tuple(out_avals),
                in_names=tuple(all_names),
                out_names=tuple(out_names),
                lowering_input_output_aliases=(),
                sim_require_finite=True,
                sim_require_nnan=True,
                nc=nc,
            )
            return tuple(outs)

        devices = jax.devices()[:N_CORES]
        assert len(devices) == N_CORES
        mesh = Mesh(np.asarray(devices), ("core",))
        nops = n_params + len(out_names)
        # AllReduce'd outputs are identical on every core: declare them
        # replicated so np.asarray pulls one replica in a single transfer
        out_spec = PartitionSpec() if OUTPUT_RS else PartitionSpec("core")
        self.fn = jax.jit(
            shard_map(_body, mesh=mesh,
                      in_specs=(PartitionSpec("core"),) * nops,
                      out_specs=(out_spec,) * len(out_names),
                      check_rep=False),
            keep_unused=True,
        )
        from jax.sharding import NamedSharding
        self._sh = NamedSharding(mesh, PartitionSpec("core"))
        self._put = jax.device_put
        self.zeros = [jax.device_put(z, self._sh) for z in zero_outs]
        self.consts = {}

    def set_constants(self, const_map):
        """Device-resident geometry-derived tables (idx/wts). These are
        functions of the compile-time geometry only - the same class of
        constants as the baked slot shapes - so they live on device like
        NEFF Const tensors would; per-call uploads carry only the
        data-dependent tensors (rows/scl)."""
        self.consts = {n: self._put(a, self._sh) for n, a in const_map.items()}

    def __call__(self, in_map):
        outs = self.fn(*[in_map.get(n, self.consts.get(n))
                         for n in self.in_names], *self.zeros)
        return {name: np.asarray(outs[i])
                for i, name in enumerate(self.out_names)}


# ----------------------------------------------------------------- packing
def _pack_inputs(idata, qdata, tabs, wtabs, slots):
    """Global (8-core concatenated) input arrays keyed by tensor name:
    (per_call, const) - per_call holds the data-dependent tensors
    (rows*/scl), const the geometry-derived tables (idx/wts).
    wtabs is [255, 6, Z] (fp16 mode) or [255, 8, Z] (int8 mode)."""
    NS = len(slots)
    NTAB = 8 if DATA_INT8 else 6
    out = {}
    idx_g = np.zeros((N_CORES, NS, 16, 256), np.int16)
    wts_g = np.zeros((N_CORES, NS, 128, NTAB * 16), np.float16)
    scl_g = np.zeros((N_CORES, NS, 128, 1), np.float32)
    if DATA_INT8:
        offs = [0]
        for sl in slots:
            offs.append(offs[-1] + sl['ext'] * sl['W'] * 4)
        scl_off = offs[-1]
        TOTAL = scl_off + NS * 256
        blob = np.zeros((N_CORES, TOTAL), np.int8)
    for k, sl in enumerate(slots):
        ext, toff, W = sl['ext'], sl['toff'], sl['W']
        rows = np.zeros((N_CORES, ext, W * (4 if DATA_INT8 else 2)),
                        np.int8 if DATA_INT8 else np.float16)
        for c in range(N_CORES):
            d = sl['deltas'][c]
            if d == DUMMY:
                continue
            j = d + 127
            lo, hi = _win(tabs, d)
            Wd = hi - lo + 1
            if d >= 0:
                ts = np.arange(d, T)
            else:
                ts = np.arange(0, T + d)
            es = ts - d
            ps = ts - toff
            I = idata[ts, es, lo:lo + Wd]
            Q = qdata[ts, es, lo:lo + Wd]
            if DATA_INT8:
                s = np.maximum(np.abs(I).max(axis=1), np.abs(Q).max(axis=1))
                s = np.maximum(s / np.float32(127.0), np.float32(1e-30))
                qI = np.zeros((len(ts), 2 * W), np.int8)
                qQ = np.zeros((len(ts), 2 * W), np.int8)
                qI[:, :Wd] = np.rint(I / s[:, None]).astype(np.int8)
                qQ[:, :Wd] = np.rint(Q / s[:, None]).astype(np.int8)
                quad = np.zeros((len(ts), W, 4), np.int8)
                quad[:, :, 0] = qI[:, 0::2]
                quad[:, :, 1] = qQ[:, 0::2]
                quad[:, :, 2] = qI[:, 1::2]
                quad[:, :, 3] = qQ[:, 1::2]
                rows[c, ps] = quad.reshape(len(ts), W * 4)
                scl_g[c, k, ps, 0] = s
                i0c = np.clip(tabs['i0'][j], lo, hi)
                i1c = np.clip(tabs['i0'][j] + 1, lo, hi)
                idx0 = ((i0c - lo) >> 1).astype(np.int16)
                idx1 = ((i1c - lo) >> 1).astype(np.int16)
            else:
                rows[c, ps, 0:2 * Wd:2] = I
                rows[c, ps, 1:2 * Wd:2] = Q
                idx0 = np.clip(tabs['i0'][j] - lo, 0, W - 1).astype(np.int16)
                idx1 = np.clip(tabs['i0'][j] + 1 - lo,
                               0, W - 1).astype(np.int16)
            idx_g[c, k, :, 0:128] = idx0.reshape(128, 16).T
            idx_g[c, k, :, 128:256] = idx1.reshape(128, 16).T
            wts_g[c, k] = wtabs[j].reshape(NTAB, NBLK, 128) \
                .transpose(2, 0, 1).reshape(128, NTAB * 16).astype(np.float16)
        if DATA_INT8:
            blob[:, offs[k]:offs[k + 1]] = rows.reshape(N_CORES, -1)
        else:
            out[f"rows{k:02d}"] = rows.reshape(N_CORES * ext, -1)
    if DATA_INT8:
        scl16 = scl_g.reshape(N_CORES, NS, 128).astype(np.float16)
        blob[:, scl_off:] = scl16.view(np.int8).reshape(N_CORES, NS * 256)
        out["blob"] = blob.reshape(-1)
    const = {"idx": idx_g.reshape(N_CORES * NS, 16, 256),
             "wts": wts_g.reshape(N_CORES * NS, 128, NTAB * 16)}
    return out, const


def _get_program(tabs):
    if 'prog' not in _CACHE:
        slots = build_slots(tabs)
        _CACHE['slots'] = slots
        _CACHE['prog'] = _build_program(slots)
        _CACHE['runner'] = _Runner(_CACHE['prog'])
    return _CACHE['prog'], _CACHE['slots'], _CACHE['runner']


def kernel(idata, qdata, grid, tx_ori, ele_pos, time_zero,
           fs, c, fdemod, rxfnum):
    idata = _f32(idata); qdata = _f32(qdata)
    tabs = compute_tables(grid, tx_ori, ele_pos, time_zero,
                          fs, c, fdemod, rxfnum)
    wtabs = (build_weight_tables16(tabs) if DATA_INT8
             else build_weight_tables(tabs))
    nc, slots, runner = _get_program(tabs)
    packed, const = _pack_inputs(idata, qdata, tabs, wtabs, slots)
    runner.set_constants(const)
    res = runner(packed)

    if OUTPUT_RS:
        # AllReduce'd on device; one replica: [0:128]=I sum, [128:256]=Q
        a = res["acc"].astype(np.float32)
        aI = a[0:128].reshape(128, NBLK, 128)
        aQ = a[128:256].reshape(128, NBLK, 128)
        idas = aI.transpose(1, 0, 2).reshape(Z, T).T.copy()
        qdas = aQ.transpose(1, 0, 2).reshape(Z, T).T.copy()
    else:
        idas = np.zeros((T, Z), np.float32)
        qdas = np.zeros((T, Z), np.float32)
        aI = res["accI"].astype(np.float32).reshape(N_CORES, 128, NBLK, 128)
        aQ = res["accQ"].astype(np.float32).reshape(N_CORES, 128, NBLK, 128)
        for cidx in range(N_CORES):
            idas += aI[cidx].transpose(1, 0, 2).reshape(Z, T).T
            qdas += aQ[cidx].transpose(1, 0, 2).reshape(Z, T).T
    cI, cQ = corrections(idata, qdata, tabs)
    idas += cI
    qdas += cQ
    return (idas, qdas)
